# revision 53
# baseline (speedup 1.0000x reference)
"""Trainium2 Bass kernel for nn_Encoder_55362128445616.

Transformer encoder layer: B=8, S=1024, D=512, single-head attention over
H*D=4096. Sharding: data-parallel over batch, one batch element per core,
no collectives.

Key algebraic folding (host-side, exact):
  scores = Q K^T / s = x (Wq^T Wk / s) x^T  (+ per-k bias from bq; per-q
  terms cancel in softmax), so the 4096-dim QKV contractions collapse to
  512-dim ones via M = Wq^T Wk / s and NT = (Wo Wv)^T.  LN1's affine is
  folded into W1/b1.

Perf design (vs the f32r baseline):
  - all matmul data is bf16 (same 1 cyc/row PE throughput as f32r, but
    half the DMA bytes and 2x faster LDWEIGHTS)
  - critical head loads (Mw at-chunks, x^T halves) split across the three
    DMA queues so phase 1 starts as early as possible
  - the z -> zT transpose between LN1 and ff1 runs on the DMA xbar
    (dma_start(transpose=True)) in qt-pairs on alternating queues, not PE
  - LN mean/var via one-pass bn_stats/bn_aggr on DVE; psum evacuations on
    ACT; LN2 applies on Pool (idle at the tail) to shorten the exit chain
"""

import math

import numpy as np
import ml_dtypes

# If the environment sets BASS_TRACE, bass_utils imports antenv.axon_hooks,
# which this image may lack — provide a no-op stub so plain runs never crash.
import sys as _sys
import types as _types
try:
    import antenv.axon_hooks  # noqa: F401
except ImportError:
    _m = _types.ModuleType("antenv.axon_hooks")
    _m.get_axon_ntff_profile_hook = lambda: None
    _m.set_axon_ntff_profile_hook = lambda hook: None
    _sys.modules["antenv.axon_hooks"] = _m

import concourse.bacc as bacc
import concourse.mybir as mybir
import concourse.tile as tile
from concourse.bass_utils import run_bass_kernel_spmd

B, S, D = 8, 1024, 512
NQ = S // 128   # 8 q/k tiles of 128
ND = D // 128   # 4 d tiles of 128
F32 = mybir.dt.float32
BF16 = mybir.dt.bfloat16
AF = mybir.ActivationFunctionType
ALU = mybir.AluOpType
N_WARM = 12

_BUILT = {}


def _build(fast):
    if fast in _BUILT:
        return _BUILT[fast]

    nc = bacc.Bacc("TRN2", target_bir_lowering=False, debug=False, num_devices=B)

    def din(name, shape, dt=BF16):
        return nc.dram_tensor(name, shape, dt, kind="ExternalInput").ap()

    # all big inputs are pre-arranged on host to the exact SBUF layout so
    # every load is a contiguous DMA at max HBM rate
    xT_d = din("xT", [4, 128, ND * 256])  # x[b].T as [q-quarter][p][bt*256]
    xn_d = din("xn", [128, NQ * D])       # x[b] as [p][kt*D]
    M_d = din("Mw", [128, ND * D])        # Wq^T Wk / sqrt(D), [p][at][bt][c]
    NT_d = din("NT", [128, ND * D])       # [p][bt][n]
    W1gT_d = din("W1gT", [128, ND * D])
    W2T_d = din("W2T", [128, ND * D])
    # packed per-partition columns: [0:8]=abias, [8:12]=c1, [12:13]=eps
    sm_d = din("smalls", [128, 16], F32)
    id_d = din("ident", [128, 128])       # bf16 identity for psum residual
    if not fast:
        xres_d = din("xres", [S, D], F32)     # x[b] + (Wo@bv + bo)
        C2_d = din("C2", [1, D])              # b2 + be0 (bf16)
        g0b_d = din("g0b", [128, D], F32)
        g1b_d = din("g1b", [128, D], F32)
        be1b_d = din("be1b", [128, D], F32)
    out_d = nc.dram_tensor("out", [S, D], F32, kind="ExternalOutput").ap()

    with tile.TileContext(nc) as tc:
        with (
            tc.tile_pool(name="res", bufs=1) as res,
            tc.tile_pool(name="work", bufs=2) as work,
            tc.tile_pool(name="small", bufs=8) as small,
            tc.tile_pool(name="psA", bufs=4, space="PSUM") as psA,
            tc.tile_pool(name="psS", bufs=2, space="PSUM") as psS,
        ):
            # ---- resident loads, split across the three DMA queues so the
            # phase-1 critical path (Mw at01 + xT h0) lands in parallel ----
            xT = res.tile([128, ND, S], BF16)
            Mw = res.tile([128, ND, ND, 128], BF16)   # [p][at][bt][c]
            Mv = M_d.rearrange("p (a t c) -> p a t c", t=ND, c=128)

            # x^T arrives in 8 independent 128KB pieces (quarter x bt-half)
            # on sync+gpsimd, and Mw in 3 at-chunks on scalar, so phase 1
            # starts on quarter 0 while the rest stream in behind it
            for qq in range(4):
                xv = xT_d[qq].rearrange("p (t c) -> p t c", c=256)
                nc.sync.dma_start(
                    xT[:, 0:2, qq * 256:(qq + 1) * 256], xv[:, 0:2, :])
                nc.gpsimd.dma_start(
                    xT[:, 2:4, qq * 256:(qq + 1) * 256], xv[:, 2:4, :])
            nc.scalar.dma_start(Mw[:, 0:1], Mv[:, 0:1])
            nc.scalar.dma_start(Mw[:, 1:2], Mv[:, 1:2])
            nc.scalar.dma_start(Mw[:, 2:4], Mv[:, 2:4])
            sm = res.tile([128, 16], F32)
            nc.scalar.dma_start(sm[:], sm_d[:])
            ident = res.tile([128, 128], BF16)
            nc.scalar.dma_start(ident[:], id_d[:])
            xn = res.tile([128, NQ, D], BF16)
            nc.gpsimd.dma_start(xn[:], xn_d.rearrange("p (t n) -> p t n", n=D))
            NTw = res.tile([128, ND, D], BF16)
            nc.gpsimd.dma_start(NTw[:], NT_d.rearrange("p (t n) -> p t n", n=D))
            W1gT = res.tile([128, ND, D], BF16)
            nc.scalar.dma_start(W1gT[:], W1gT_d.rearrange("p (t n) -> p t n", n=D))
            W2T = res.tile([128, ND, D], BF16)
            nc.scalar.dma_start(W2T[:], W2T_d.rearrange("p (t n) -> p t n", n=D))
            if not fast:
                xres = res.tile([128, NQ, D], F32)
                nc.gpsimd.dma_start(xres[:], xres_d.rearrange("(t p) n -> p t n", p=128))
                C2 = res.tile([1, D], BF16)
                nc.gpsimd.dma_start(C2[:], C2_d[:])
                g0b = res.tile([128, D], F32)
                nc.gpsimd.dma_start(g0b[:], g0b_d[:])
                g1b = res.tile([128, D], F32)
                nc.gpsimd.dma_start(g1b[:], g1b_d[:])
                be1b = res.tile([128, D], F32)
                nc.gpsimd.dma_start(be1b[:], be1b_d[:])

            abias = sm[:, 0:8]
            c1 = sm[:, 8:12]
            epsT = sm[:, 12:13]

            # device-built constants (no DMA)
            onesb = res.tile([128, 2], BF16)
            nc.vector.memset(onesb[:], 1.0)
            if not fast:
                onesr = res.tile([1, 128], BF16)
                nc.vector.memset(onesr[:], 1.0)

            # HAM warm-up on an uninitialized scratch tile: no input deps, so
            # the PE starts (and its p-state ramp clock) while DMAs land.
            wtile = res.tile([128, 128], BF16)
            nc.vector.memset(wtile[:], 0.0)
            for w in range(N_WARM):
                psw = psA.tile([128, 128], F32, tag="a", name=f"psw{w}")
                nc.tensor.matmul(psw[:], wtile[:], wtile[:], start=True, stop=True)

            # big SBUF intermediates (all bf16)
            IN2 = res.tile([128, ND, S], BF16)      # (x M)^T
            PT = res.tile([128, NQ, S], BF16)       # exp(scores^T)
            ST = res.tile([128, ND, S], BF16)       # sdpa^T
            # zT in qt-major blocks: [p][qt][dt][c]; a qt-pair DMA-transpose
            # writes one contiguous [128, 1024] span
            zT = res.tile([128, NQ, ND, 128], BF16)
            ff1T = res.tile([128, ND, S], BF16)
            z = res.tile([128, NQ, D], BF16)        # LN1 out
            recip = res.tile([128, 2 * NQ], F32)

            # ---- phase 1: IN2[a, q] = sum_b M[b, a] xT[b, q] ----
            # quarter-granular so compute starts as soon as the first
            # 256-column piece of x^T has landed
            for qq in range(4):
                for at in range(ND):
                    ps = psA.tile([128, 256], F32, tag="a", name="ps_p1")
                    for bt in range(ND):
                        nc.tensor.matmul(
                            ps[:],
                            Mw[:, at, bt, :],
                            xT[:, bt, qq * 256:(qq + 1) * 256],
                            start=(bt == 0), stop=(bt == ND - 1),
                        )
                    nc.scalar.copy(IN2[:, at, qq * 256:(qq + 1) * 256], ps[:])

            # ---- phase 2: scoresT[k, q] = sum_a x[k, a] IN2[a, q]; PT = exp ----
            for kt in range(NQ):
                ps = psS.tile([128, 1024], F32, tag="s")
                for qc in range(2):
                    for at in range(ND):
                        nc.tensor.matmul(
                            ps[:, qc * 512:(qc + 1) * 512],
                            xT[:, at, kt * 128:(kt + 1) * 128],
                            IN2[:, at, qc * 512:(qc + 1) * 512],
                            start=(at == 0), stop=(at == ND - 1),
                        )
                bias = 0.0 if fast else abias[:, kt:kt + 1]
                nc.scalar.activation(PT[:, kt, :], ps[:], AF.Exp, bias=bias)

            # softmax denominator: DVE tree-sums the 8 k-tiles (pairwise),
            # then one tiny ones-matmul per q-tile flips [k-part, q] to
            # [q-part, 1].
            def tree(qc):
                qs = slice(qc * 512, (qc + 1) * 512)
                lvl1 = []
                for i in range(4):
                    t = work.tile([128, 512], BF16, tag="tr", bufs=6, name=f"t{qc}{i}")
                    nc.vector.tensor_add(t[:], PT[:, 2 * i, qs], PT[:, 2 * i + 1, qs])
                    lvl1.append(t)
                u0 = work.tile([128, 512], BF16, tag="tr", bufs=6, name=f"u{qc}0")
                nc.vector.tensor_add(u0[:], lvl1[0][:], lvl1[1][:])
                u1 = work.tile([128, 512], BF16, tag="tr", bufs=6, name=f"u{qc}1")
                nc.vector.tensor_add(u1[:], lvl1[2][:], lvl1[3][:])
                dacc = work.tile([128, 512], BF16, tag="dacc", bufs=2, name=f"dacc{qc}")
                nc.vector.tensor_add(dacc[:], u0[:], u1[:])
                return dacc

            def denoms(qc, dacc):
                dps = psA.tile([128, 512], F32, tag="a", name=f"dps{qc}")
                for ql in range(4):
                    nc.tensor.matmul(
                        dps[:, 2 * ql:2 * ql + 2],
                        dacc[:, ql * 128:(ql + 1) * 128],
                        onesb[:],
                        start=True, stop=True,
                    )
                nc.vector.reciprocal(recip[:, qc * 8:(qc + 1) * 8], dps[:, 0:8])

            # ---- phase 3: ST[d, q] = sum_k x[k, d] PT[k, q] ----
            def st_chunk(qc):
                for dt in range(ND):
                    ps = psA.tile([128, 512], F32, tag="a", name="ps_st")
                    for kt in range(NQ):
                        nc.tensor.matmul(
                            ps[:],
                            xn[:, kt, dt * 128:(dt + 1) * 128],
                            PT[:, kt, qc * 512:(qc + 1) * 512],
                            start=(kt == 0), stop=(kt == NQ - 1),
                        )
                    nc.scalar.copy(ST[:, dt, qc * 512:(qc + 1) * 512], ps[:])

            # ---- phase 4: mha + residual + LN1 stats per q-tile ----
            def mha_stats(qts):
                x1s, lns = [], []
                for qt in qts:
                    ps = psA.tile([128, 512], F32, tag="a", name="ps_mha")
                    for dt in range(ND):
                        nc.tensor.matmul(
                            ps[:],
                            ST[:, dt, qt * 128:(qt + 1) * 128],
                            NTw[:, dt, :],
                            start=(dt == 0), stop=(dt == ND - 1),
                        )
                    x1 = work.tile([128, D], BF16, tag="x1", bufs=NQ, name=f"x1_{qt}")
                    resid = xn[:, qt, :] if fast else xres[:, qt, :]
                    # alternate the stats chain between DVE (bn_stats) and
                    # ACT (square+accum) lanes so neither engine saturates;
                    # qt>=4 all on ACT so the DVE applies (which gate the
                    # zT transposes for ff1 qc1) clear sooner
                    on_act = qt % 2 == 1 or qt >= 4
                    s1 = (small.tile([128, 1], F32, tag="s1", name=f"s1_{qt}")
                          if on_act else None)
                    nc.vector.scalar_tensor_tensor(
                        x1[:], ps[:], recip[:, 2 * qt:2 * qt + 1], resid,
                        op0=ALU.mult, op1=ALU.add,
                        accum_out=s1[:] if on_act else None,
                    )
                    x1s.append(x1)
                    if on_act:
                        lns.append(_ln_stats_act(nc, small, work, x1, s1, epsT))
                    else:
                        lns.append(_ln_stats(nc, small, x1[:], epsT))
                return x1s, lns

            # ---- LN1 apply; DMA-xbar transpose z -> zT in qt pairs ----
            def apply_xpose(qts, x1s, lns):
                for i, qt in enumerate(qts):
                    _ln_apply(nc.vector, x1s[i][:], lns[i], z[:, qt, :])
                    # all transposes on the sync queue: it is idle mid-body,
                    # while a transpose on the scalar queue would block the
                    # ACT engine behind it (psum evacuations, relu)
                    if qt % 2 == 1:
                        q0 = qt - 1
                        nc.sync.dma_start(
                            zT[:, q0:q0 + 2], z[:, q0:q0 + 2, :],
                            transpose=True)

            # ---- phase 5: ff1 ----
            def ff1_chunk(qc):
                for et in range(ND):
                    ps = psA.tile([128, 512], F32, tag="a", name="ps_ff1")
                    for dt in range(ND):
                        nc.tensor.matmul(
                            ps[:],
                            W1gT[:, dt, et * 128:(et + 1) * 128],
                            zT[:, qc * 4:(qc + 1) * 4, dt, :],
                            start=(dt == 0), stop=(dt == ND - 1),
                        )
                    bias = 0.0 if fast else c1[:, et:et + 1]
                    nc.scalar.activation(
                        ff1T[:, et, qc * 512:(qc + 1) * 512], ps[:],
                        AF.Relu, bias=bias,
                    )

            # ---- phase 6: ff2 + residual + LN2 stats ----
            # fast path: the z residual is added INTO the psum by one extra
            # identity matmul, so stats and apply read the psum directly —
            # no DVE stt at all (and f32 residual precision for free)
            def ff2_stats(qts):
                rs, lns2 = [], []
                if fast:
                    # interleave the q-tiles' et-accumulations so the last
                    # relu evacuation's latency is covered by the other
                    # tile's matmuls (removes the ff1->ff2 boundary stall)
                    pss = [psA.tile([128, 512], F32, tag="a", name=f"ps_ff2_{qt}")
                           for qt in qts]
                    for et in range(ND):
                        for i, qt in enumerate(qts):
                            nc.tensor.matmul(
                                pss[i][:],
                                ff1T[:, et, qt * 128:(qt + 1) * 128],
                                W2T[:, et, :],
                                start=(et == 0), stop=False,
                            )
                    for i, qt in enumerate(qts):
                        nc.tensor.matmul(pss[i][:], ident[:], z[:, qt, :],
                                         start=False, stop=True)
                    for i, qt in enumerate(qts):
                        rs.append(pss[i])
                        lns2.append(_ln_stats(nc, small, pss[i][:], epsT))
                    return rs, lns2
                for qt in qts:
                    ps = psA.tile([128, 512], F32, tag="a", name="ps_ff2")
                    for et in range(ND):
                        nc.tensor.matmul(
                            ps[:],
                            ff1T[:, et, qt * 128:(qt + 1) * 128],
                            W2T[:, et, :],
                            start=(et == 0), stop=False,
                        )
                    if True:
                        nc.tensor.matmul(ps[:], onesr[:], C2[:], start=False, stop=True)
                        r = work.tile([128, D], BF16, tag="r", bufs=NQ, name=f"r_{qt}")
                        hres = work.tile([128, D], F32, tag="hres")
                        nc.vector.tensor_mul(hres[:], z[:, qt, :], g0b[:])
                        nc.vector.scalar_tensor_tensor(
                            r[:], ps[:], 1.0, hres[:],
                            op0=ALU.mult, op1=ALU.add,
                        )
                        rs.append(r)
                        lns2.append(_ln_stats(nc, small, r[:], epsT))
                return rs, lns2

            def ln2_out(qts, rs, lns2, split=False):
                for i, qt in enumerate(qts):
                    od = out_d.rearrange("(t p) n -> p t n", p=128)[:, qt, :]
                    o = work.tile([128, D], F32, tag="o", bufs=3, name=f"o_{qt}")
                    # all stores go on the sync queue: a store issued on the
                    # scalar queue blocks the ACT engine behind it, delaying
                    # the tail sqrt chains
                    if fast and split:
                        # halves pipeline the apply with the store at the tail
                        for h in range(2):
                            hs = slice(h * 256, (h + 1) * 256)
                            _ln_apply(nc.vector, rs[i][:, hs], lns2[i], o[:, hs])
                            nc.sync.dma_start(od[:, hs], o[:, hs])
                    elif fast:
                        _ln_apply(nc.vector, rs[i][:], lns2[i], o[:])
                        nc.sync.dma_start(od, o[:])
                    else:
                        z2 = work.tile([128, D], F32, tag="z2")
                        _ln_apply(nc.vector, rs[i][:], lns2[i], z2[:])
                        nc.vector.tensor_mul(o[:], z2[:], g1b[:])
                        nc.vector.tensor_add(o[:], o[:], be1b[:])
                        nc.sync.dma_start(od, o[:])

            # ---- emission order = near-execution order per engine ----
            st_chunk(0)
            d0 = tree(0)
            denoms(0, d0)
            a0, l0 = mha_stats([0, 1, 2, 3])
            st_chunk(1)
            d1 = tree(1)
            denoms(1, d1)
            apply_xpose([0, 1, 2, 3], a0, l0)
            a1, l1 = mha_stats([4, 5, 6, 7])
            apply_xpose([4, 5, 6, 7], a1, l1)
            ff1_chunk(0)
            r01, l01 = ff2_stats([0, 1])
            ln2_out([0, 1], r01, l01)
            r23, l23 = ff2_stats([2, 3])
            ln2_out([2, 3], r23, l23)
            ff1_chunk(1)
            r45, l45 = ff2_stats([4, 5])
            ln2_out([4, 5], r45, l45)
            r6, l6 = ff2_stats([6])
            ln2_out([6], r6, l6, split=True)
            r7, l7 = ff2_stats([7])
            ln2_out([7], r7, l7, split=True)

    nc.compile()
    _BUILT[fast] = (nc,)
    return _BUILT[fast]


def _ln_stats(nc, small, ap, epsT):
    """One-pass LN stats on DVE: bn_stats/bn_aggr give (mean, var);
    rstd = 1/sqrt(var + eps).  Returns (-mean, rstd) [128,1] tiles.
    Mean is negated so the apply can use the DVE add+mult fast path
    (op0=subtract falls off the fast uop table)."""
    bns = small.tile([128, 6], F32, tag="bns")
    nc.vector.bn_stats(bns[:], ap)
    mv = small.tile([128, 2], F32, tag="mv")
    nc.vector.bn_aggr(mv[:], bns[:])
    nm = small.tile([128, 1], F32, tag="nm")
    nc.vector.tensor_scalar_mul(nm[:], mv[:, 0:1], -1.0)
    stdv = small.tile([128, 1], F32, tag="stdv")
    nc.scalar.activation(stdv[:], mv[:, 1:2], AF.Sqrt, bias=epsT[:])
    rstd = small.tile([128, 1], F32, tag="rstd")
    nc.vector.reciprocal(rstd[:], stdv[:])
    return nm, rstd


def _ln_stats_act(nc, small, work, x1, s1, epsT):
    """ACT-lane LN stats: s1 = row-sum from the caller's accum_out;
    sum-of-squares via ACT Square+accum; var = E[x^2] - mean^2."""
    sq = work.tile([128, D], BF16, tag="sq")
    sqs = small.tile([128, 1], F32, tag="sqs")
    nc.scalar.activation(sq[:], x1[:], AF.Square, bias=0.0, accum_out=sqs[:])
    nm = small.tile([128, 1], F32, tag="nm")
    nc.scalar.mul(nm[:], s1[:], -1.0 / D)
    m2e = small.tile([128, 1], F32, tag="m2e")
    nc.vector.tensor_mul(m2e[:], nm[:], nm[:])
    nc.vector.tensor_scalar(m2e[:], m2e[:], -1.0, 1e-5,
                            op0=ALU.mult, op1=ALU.add)
    stdv = small.tile([128, 1], F32, tag="stdv")
    nc.scalar.activation(stdv[:], sqs[:], AF.Sqrt, scale=1.0 / D, bias=m2e[:])
    rstd = small.tile([128, 1], F32, tag="rstd")
    nc.vector.reciprocal(rstd[:], stdv[:])
    return nm, rstd


def _ln_apply(eng, ap, stats, out_ap):
    nm, rstd = stats
    # (x - mean) * rstd as one op with two per-partition scalars
    eng.tensor_scalar(out_ap, ap, nm[:], rstd[:],
                      op0=ALU.add, op1=ALU.mult)


def _prepare_in_maps(inputs):
    f64 = np.float64
    bf = ml_dtypes.bfloat16
    g = {k: np.asarray(v) for k, v in inputs.items()}
    x = g["x"].astype(f64)
    Wq, Wk, Wv = g["Wq"].astype(f64), g["Wk"].astype(f64), g["Wv"].astype(f64)
    Wo, W1, W2 = g["Wo"].astype(f64), g["W1"].astype(f64), g["W2"].astype(f64)
    bq, bk, bv, bo = g["bq"].astype(f64), g["bk"].astype(f64), g["bv"].astype(f64), g["bo"].astype(f64)
    b1, b2 = g["b1"].astype(f64), g["b2"].astype(f64)
    g0, be0, g1, be1 = g["g0"].astype(f64), g["be0"].astype(f64), g["g1"].astype(f64), g["be1"].astype(f64)

    fast = (
        not np.any(bq) and not np.any(bk) and not np.any(bv) and not np.any(bo)
        and not np.any(b1) and not np.any(b2) and not np.any(be0) and not np.any(be1)
        and bool(np.all(g0 == 1.0)) and bool(np.all(g1 == 1.0))
    )

    s = math.sqrt(D)
    f32 = lambda a: np.ascontiguousarray(a, dtype=np.float32)
    b16 = lambda a: np.ascontiguousarray(a, dtype=np.float32).astype(bf)

    def wlay(W):  # [512, 512] -> SBUF layout [128, bt*512 + n], bf16
        return b16(W.reshape(ND, 128, D).transpose(1, 0, 2).reshape(128, ND * D))

    def wlay_at(W):  # [512, 512] -> [128, at*512 + bt*128 + c], bf16
        return b16(W.reshape(ND, 128, ND, 128).transpose(1, 2, 0, 3)
                   .reshape(128, ND * D))

    Mw = wlay_at(Wq.T @ Wk / s)
    NT = wlay((Wo @ Wv).T)
    W1gT = wlay((W1 * g0[None, :]).T)
    W2T = wlay(W2.T)
    c1 = f32(b1 + W1 @ be0)
    wbo = Wo @ bv + bo
    vk = Wk.T @ bq / s

    shared = dict(Mw=Mw, NT=NT, W1gT=W1gT, W2T=W2T,
                  ident=np.eye(128).astype(bf))
    if not fast:
        shared["C2"] = b16(b2 + be0).reshape(1, D)
        shared["g0b"] = f32(np.broadcast_to(g0, (128, D)))
        shared["g1b"] = f32(np.broadcast_to(g1, (128, D)))
        shared["be1b"] = f32(np.broadcast_to(be1, (128, D)))

    in_maps = []
    for b in range(B):
        xb = x[b]
        m = dict(shared)
        xTf = xb.T.reshape(ND, 128, 4, 256).transpose(2, 1, 0, 3)
        m["xT"] = b16(xTf.reshape(4, 128, ND * 256))
        m["xn"] = b16(xb.reshape(NQ, 128, D).transpose(1, 0, 2).reshape(128, NQ * D))
        smalls = np.zeros((128, 16), np.float32)
        smalls[:, 0:8] = f32(xb @ vk).reshape(8, 128).T
        smalls[:, 8:12] = c1.reshape(4, 128).T
        smalls[:, 12:13] = 1e-5
        m["smalls"] = smalls
        if not fast:
            m["xres"] = f32(xb + wbo[None, :])
        in_maps.append(m)
    return fast, in_maps


def _run(inputs, trace=False):
    fast, in_maps = _prepare_in_maps(inputs)
    (nc,) = _build(fast)
    res = run_bass_kernel_spmd(nc, in_maps, core_ids=list(range(B)), trace=trace)
    out = np.stack([res.results[c]["out"] for c in range(B)]).astype(np.float32)
    return out, res


def kernel(**inputs):
    out, _ = _run(inputs, trace=False)
    return out


# revision 55
# speedup vs baseline: 1.0105x; 1.0105x over previous
"""Trainium2 Bass kernel for nn_Encoder_55362128445616.

Transformer encoder layer: B=8, S=1024, D=512, single-head attention over
H*D=4096. Sharding: data-parallel over batch, one batch element per core,
no collectives.

Key algebraic folding (host-side, exact):
  scores = Q K^T / s = x (Wq^T Wk / s) x^T  (+ per-k bias from bq; per-q
  terms cancel in softmax), so the 4096-dim QKV contractions collapse to
  512-dim ones via M = Wq^T Wk / s and NT = (Wo Wv)^T.  LN1's affine is
  folded into W1/b1.

Perf design (vs the f32r baseline):
  - all matmul data is bf16 (same 1 cyc/row PE throughput as f32r, but
    half the DMA bytes and 2x faster LDWEIGHTS)
  - critical head loads (Mw at-chunks, x^T halves) split across the three
    DMA queues so phase 1 starts as early as possible
  - the z -> zT transpose between LN1 and ff1 runs on the DMA xbar
    (dma_start(transpose=True)) in qt-pairs on alternating queues, not PE
  - LN mean/var via one-pass bn_stats/bn_aggr on DVE; psum evacuations on
    ACT; LN2 applies on Pool (idle at the tail) to shorten the exit chain
"""

import math

import numpy as np
import ml_dtypes

# If the environment sets BASS_TRACE, bass_utils imports antenv.axon_hooks,
# which this image may lack — provide a no-op stub so plain runs never crash.
import sys as _sys
import types as _types
try:
    import antenv.axon_hooks  # noqa: F401
except ImportError:
    _m = _types.ModuleType("antenv.axon_hooks")
    _m.get_axon_ntff_profile_hook = lambda: None
    _m.set_axon_ntff_profile_hook = lambda hook: None
    _sys.modules["antenv.axon_hooks"] = _m

import concourse.bacc as bacc
import concourse.mybir as mybir
import concourse.tile as tile
from concourse.bass_utils import run_bass_kernel_spmd

B, S, D = 8, 1024, 512
NQ = S // 128   # 8 q/k tiles of 128
ND = D // 128   # 4 d tiles of 128
F32 = mybir.dt.float32
BF16 = mybir.dt.bfloat16
AF = mybir.ActivationFunctionType
ALU = mybir.AluOpType
N_WARM = 24

_BUILT = {}


def _build(fast):
    if fast in _BUILT:
        return _BUILT[fast]

    nc = bacc.Bacc("TRN2", target_bir_lowering=False, debug=False, num_devices=B)

    def din(name, shape, dt=BF16):
        return nc.dram_tensor(name, shape, dt, kind="ExternalInput").ap()

    # all big inputs are pre-arranged on host to the exact SBUF layout so
    # every load is a contiguous DMA at max HBM rate
    xT_d = din("xT", [4, 128, ND * 256])  # x[b].T as [q-quarter][p][bt*256]
    xn_d = din("xn", [128, NQ * D])       # x[b] as [p][kt*D]
    M_d = din("Mw", [128, ND * D])        # Wq^T Wk / sqrt(D), [p][at][bt][c]
    NT_d = din("NT", [128, ND * D])       # [p][bt][n]
    W1gT_d = din("W1gT", [128, ND * D])
    W2T_d = din("W2T", [128, ND * D])
    # packed per-partition columns: [0:8]=abias, [8:12]=c1, [12:13]=eps
    sm_d = din("smalls", [128, 16], F32)
    id_d = din("ident", [128, 128])       # bf16 identity for psum residual
    if not fast:
        xres_d = din("xres", [S, D], F32)     # x[b] + (Wo@bv + bo)
        C2_d = din("C2", [1, D])              # b2 + be0 (bf16)
        g0b_d = din("g0b", [128, D], F32)
        g1b_d = din("g1b", [128, D], F32)
        be1b_d = din("be1b", [128, D], F32)
    out_d = nc.dram_tensor("out", [S, D], F32, kind="ExternalOutput").ap()

    with tile.TileContext(nc) as tc:
        with (
            tc.tile_pool(name="res", bufs=1) as res,
            tc.tile_pool(name="work", bufs=2) as work,
            tc.tile_pool(name="small", bufs=8) as small,
            tc.tile_pool(name="psA", bufs=4, space="PSUM") as psA,
            tc.tile_pool(name="psS", bufs=2, space="PSUM") as psS,
        ):
            # ---- resident loads, split across the three DMA queues so the
            # phase-1 critical path (Mw at01 + xT h0) lands in parallel ----
            xT = res.tile([128, ND, S], BF16)
            Mw = res.tile([128, ND, ND, 128], BF16)   # [p][at][bt][c]
            Mv = M_d.rearrange("p (a t c) -> p a t c", t=ND, c=128)

            # x^T arrives in 8 independent 128KB pieces (quarter x bt-half)
            # on sync+gpsimd, and Mw in 3 at-chunks on scalar, so phase 1
            # starts on quarter 0 while the rest stream in behind it
            for qq in range(4):
                xv = xT_d[qq].rearrange("p (t c) -> p t c", c=256)
                nc.sync.dma_start(
                    xT[:, 0:2, qq * 256:(qq + 1) * 256], xv[:, 0:2, :])
                nc.gpsimd.dma_start(
                    xT[:, 2:4, qq * 256:(qq + 1) * 256], xv[:, 2:4, :])
            nc.scalar.dma_start(Mw[:, 0:1], Mv[:, 0:1])
            nc.scalar.dma_start(Mw[:, 1:2], Mv[:, 1:2])
            nc.scalar.dma_start(Mw[:, 2:4], Mv[:, 2:4])
            sm = res.tile([128, 16], F32)
            nc.scalar.dma_start(sm[:], sm_d[:])
            ident = res.tile([128, 128], BF16)
            nc.scalar.dma_start(ident[:], id_d[:])
            xn = res.tile([128, NQ, D], BF16)
            nc.gpsimd.dma_start(xn[:], xn_d.rearrange("p (t n) -> p t n", n=D))
            NTw = res.tile([128, ND, D], BF16)
            nc.gpsimd.dma_start(NTw[:], NT_d.rearrange("p (t n) -> p t n", n=D))
            W1gT = res.tile([128, ND, D], BF16)
            nc.scalar.dma_start(W1gT[:], W1gT_d.rearrange("p (t n) -> p t n", n=D))
            W2T = res.tile([128, ND, D], BF16)
            nc.scalar.dma_start(W2T[:], W2T_d.rearrange("p (t n) -> p t n", n=D))
            if not fast:
                xres = res.tile([128, NQ, D], F32)
                nc.gpsimd.dma_start(xres[:], xres_d.rearrange("(t p) n -> p t n", p=128))
                C2 = res.tile([1, D], BF16)
                nc.gpsimd.dma_start(C2[:], C2_d[:])
                g0b = res.tile([128, D], F32)
                nc.gpsimd.dma_start(g0b[:], g0b_d[:])
                g1b = res.tile([128, D], F32)
                nc.gpsimd.dma_start(g1b[:], g1b_d[:])
                be1b = res.tile([128, D], F32)
                nc.gpsimd.dma_start(be1b[:], be1b_d[:])

            abias = sm[:, 0:8]
            c1 = sm[:, 8:12]
            epsT = sm[:, 12:13]

            # device-built constants (no DMA)
            onesb = res.tile([128, 2], BF16)
            nc.vector.memset(onesb[:], 1.0)
            if not fast:
                onesr = res.tile([1, 128], BF16)
                nc.vector.memset(onesr[:], 1.0)

            # HAM warm-up on an uninitialized scratch tile: no input deps, so
            # the PE starts (and its p-state ramp clock) while DMAs land.
            wtile = res.tile([128, 128], BF16)
            nc.vector.memset(wtile[:], 0.0)
            for w in range(N_WARM):
                psw = psA.tile([128, 128], F32, tag="a", name=f"psw{w}")
                nc.tensor.matmul(psw[:], wtile[:], wtile[:], start=True, stop=True)

            # big SBUF intermediates (all bf16)
            IN2 = res.tile([128, ND, S], BF16)      # (x M)^T
            PT = res.tile([128, NQ, S], BF16)       # exp(scores^T)
            ST = res.tile([128, ND, S], BF16)       # sdpa^T
            # zT in qt-major blocks: [p][qt][dt][c]; a qt-pair DMA-transpose
            # writes one contiguous [128, 1024] span
            zT = res.tile([128, NQ, ND, 128], BF16)
            ff1T = res.tile([128, ND, S], BF16)
            z = res.tile([128, NQ, D], BF16)        # LN1 out
            recip = res.tile([128, 2 * NQ], F32)

            # ---- phase 1: IN2[a, q] = sum_b M[b, a] xT[b, q] ----
            # 512-column chunks (matmuls have a ~213ns instruction floor on
            # HW, so smaller chunks don't pay); the quartered loads mean the
            # qc0 half only waits for x^T quarters 0+1
            for qc in range(2):
                for at in range(ND):
                    ps = psA.tile([128, 512], F32, tag="a", name="ps_p1")
                    for bt in range(ND):
                        nc.tensor.matmul(
                            ps[:],
                            Mw[:, at, bt, :],
                            xT[:, bt, qc * 512:(qc + 1) * 512],
                            start=(bt == 0), stop=(bt == ND - 1),
                        )
                    nc.scalar.copy(IN2[:, at, qc * 512:(qc + 1) * 512], ps[:])

            # ---- phase 2: scoresT[k, q] = sum_a x[k, a] IN2[a, q]; PT = exp ----
            for kt in range(NQ):
                ps = psS.tile([128, 1024], F32, tag="s")
                for qc in range(2):
                    for at in range(ND):
                        nc.tensor.matmul(
                            ps[:, qc * 512:(qc + 1) * 512],
                            xT[:, at, kt * 128:(kt + 1) * 128],
                            IN2[:, at, qc * 512:(qc + 1) * 512],
                            start=(at == 0), stop=(at == ND - 1),
                        )
                bias = 0.0 if fast else abias[:, kt:kt + 1]
                nc.scalar.activation(PT[:, kt, :], ps[:], AF.Exp, bias=bias)

            # softmax denominator: DVE tree-sums the 8 k-tiles (pairwise),
            # then one tiny ones-matmul per q-tile flips [k-part, q] to
            # [q-part, 1].
            def tree(qc):
                qs = slice(qc * 512, (qc + 1) * 512)
                lvl1 = []
                for i in range(4):
                    t = work.tile([128, 512], BF16, tag="tr", bufs=6, name=f"t{qc}{i}")
                    nc.vector.tensor_add(t[:], PT[:, 2 * i, qs], PT[:, 2 * i + 1, qs])
                    lvl1.append(t)
                u0 = work.tile([128, 512], BF16, tag="tr", bufs=6, name=f"u{qc}0")
                nc.vector.tensor_add(u0[:], lvl1[0][:], lvl1[1][:])
                u1 = work.tile([128, 512], BF16, tag="tr", bufs=6, name=f"u{qc}1")
                nc.vector.tensor_add(u1[:], lvl1[2][:], lvl1[3][:])
                dacc = work.tile([128, 512], BF16, tag="dacc", bufs=2, name=f"dacc{qc}")
                nc.vector.tensor_add(dacc[:], u0[:], u1[:])
                return dacc

            def denoms(qc, dacc):
                dps = psA.tile([128, 512], F32, tag="a", name=f"dps{qc}")
                for ql in range(4):
                    nc.tensor.matmul(
                        dps[:, 2 * ql:2 * ql + 2],
                        dacc[:, ql * 128:(ql + 1) * 128],
                        onesb[:],
                        start=True, stop=True,
                    )
                nc.vector.reciprocal(recip[:, qc * 8:(qc + 1) * 8], dps[:, 0:8])

            # ---- phase 3: ST[d, q] = sum_k x[k, d] PT[k, q] ----
            def st_chunk(qc):
                for dt in range(ND):
                    ps = psA.tile([128, 512], F32, tag="a", name="ps_st")
                    for kt in range(NQ):
                        nc.tensor.matmul(
                            ps[:],
                            xn[:, kt, dt * 128:(dt + 1) * 128],
                            PT[:, kt, qc * 512:(qc + 1) * 512],
                            start=(kt == 0), stop=(kt == NQ - 1),
                        )
                    nc.scalar.copy(ST[:, dt, qc * 512:(qc + 1) * 512], ps[:])

            # ---- phase 4: mha + residual + LN1 stats per q-tile ----
            def mha_stats(qts):
                x1s, lns = [], []
                for qt in qts:
                    ps = psA.tile([128, 512], F32, tag="a", name="ps_mha")
                    for dt in range(ND):
                        nc.tensor.matmul(
                            ps[:],
                            ST[:, dt, qt * 128:(qt + 1) * 128],
                            NTw[:, dt, :],
                            start=(dt == 0), stop=(dt == ND - 1),
                        )
                    x1 = work.tile([128, D], BF16, tag="x1", bufs=NQ, name=f"x1_{qt}")
                    resid = xn[:, qt, :] if fast else xres[:, qt, :]
                    # alternate the stats chain between DVE (bn_stats) and
                    # ACT (square+accum) lanes so neither engine saturates;
                    # qt>=4 all on ACT so the DVE applies (which gate the
                    # zT transposes for ff1 qc1) clear sooner
                    on_act = qt % 2 == 1 or qt >= 4
                    s1 = (small.tile([128, 1], F32, tag="s1", name=f"s1_{qt}")
                          if on_act else None)
                    nc.vector.scalar_tensor_tensor(
                        x1[:], ps[:], recip[:, 2 * qt:2 * qt + 1], resid,
                        op0=ALU.mult, op1=ALU.add,
                        accum_out=s1[:] if on_act else None,
                    )
                    x1s.append(x1)
                    if on_act:
                        lns.append(_ln_stats_act(nc, small, work, x1, s1, epsT))
                    else:
                        lns.append(_ln_stats(nc, small, x1[:], epsT))
                return x1s, lns

            # ---- LN1 apply; DMA-xbar transpose z -> zT in qt pairs ----
            def apply_xpose(qts, x1s, lns):
                for i, qt in enumerate(qts):
                    _ln_apply(nc.vector, x1s[i][:], lns[i], z[:, qt, :])
                    # all transposes on the sync queue: it is idle mid-body,
                    # while a transpose on the scalar queue would block the
                    # ACT engine behind it (psum evacuations, relu)
                    if qt % 2 == 1:
                        q0 = qt - 1
                        nc.sync.dma_start(
                            zT[:, q0:q0 + 2], z[:, q0:q0 + 2, :],
                            transpose=True)

            # ---- phase 5: ff1 ----
            def ff1_chunk(qc):
                for et in range(ND):
                    ps = psA.tile([128, 512], F32, tag="a", name="ps_ff1")
                    for dt in range(ND):
                        nc.tensor.matmul(
                            ps[:],
                            W1gT[:, dt, et * 128:(et + 1) * 128],
                            zT[:, qc * 4:(qc + 1) * 4, dt, :],
                            start=(dt == 0), stop=(dt == ND - 1),
                        )
                    bias = 0.0 if fast else c1[:, et:et + 1]
                    nc.scalar.activation(
                        ff1T[:, et, qc * 512:(qc + 1) * 512], ps[:],
                        AF.Relu, bias=bias,
                    )

            # ---- phase 6: ff2 + residual + LN2 stats ----
            # fast path: the z residual is added INTO the psum by one extra
            # identity matmul, so stats and apply read the psum directly —
            # no DVE stt at all (and f32 residual precision for free)
            def ff2_stats(qts):
                rs, lns2 = [], []
                if fast:
                    # interleave the q-tiles' et-accumulations so the last
                    # relu evacuation's latency is covered by the other
                    # tile's matmuls (removes the ff1->ff2 boundary stall)
                    pss = [psA.tile([128, 512], F32, tag="a", name=f"ps_ff2_{qt}")
                           for qt in qts]
                    for et in range(ND):
                        for i, qt in enumerate(qts):
                            nc.tensor.matmul(
                                pss[i][:],
                                ff1T[:, et, qt * 128:(qt + 1) * 128],
                                W2T[:, et, :],
                                start=(et == 0), stop=False,
                            )
                    for i, qt in enumerate(qts):
                        nc.tensor.matmul(pss[i][:], ident[:], z[:, qt, :],
                                         start=False, stop=True)
                    for i, qt in enumerate(qts):
                        rs.append(pss[i])
                        lns2.append(_ln_stats(nc, small, pss[i][:], epsT))
                    return rs, lns2
                for qt in qts:
                    ps = psA.tile([128, 512], F32, tag="a", name="ps_ff2")
                    for et in range(ND):
                        nc.tensor.matmul(
                            ps[:],
                            ff1T[:, et, qt * 128:(qt + 1) * 128],
                            W2T[:, et, :],
                            start=(et == 0), stop=False,
                        )
                    if True:
                        nc.tensor.matmul(ps[:], onesr[:], C2[:], start=False, stop=True)
                        r = work.tile([128, D], BF16, tag="r", bufs=NQ, name=f"r_{qt}")
                        hres = work.tile([128, D], F32, tag="hres")
                        nc.vector.tensor_mul(hres[:], z[:, qt, :], g0b[:])
                        nc.vector.scalar_tensor_tensor(
                            r[:], ps[:], 1.0, hres[:],
                            op0=ALU.mult, op1=ALU.add,
                        )
                        rs.append(r)
                        lns2.append(_ln_stats(nc, small, r[:], epsT))
                return rs, lns2

            def ln2_out(qts, rs, lns2, split=False):
                for i, qt in enumerate(qts):
                    od = out_d.rearrange("(t p) n -> p t n", p=128)[:, qt, :]
                    o = work.tile([128, D], F32, tag="o", bufs=3, name=f"o_{qt}")
                    # all stores go on the sync queue: a store issued on the
                    # scalar queue blocks the ACT engine behind it, delaying
                    # the tail sqrt chains
                    if fast and split:
                        # halves pipeline the apply with the store at the tail
                        for h in range(2):
                            hs = slice(h * 256, (h + 1) * 256)
                            _ln_apply(nc.vector, rs[i][:, hs], lns2[i], o[:, hs])
                            nc.sync.dma_start(od[:, hs], o[:, hs])
                    elif fast:
                        _ln_apply(nc.vector, rs[i][:], lns2[i], o[:])
                        nc.sync.dma_start(od, o[:])
                    else:
                        z2 = work.tile([128, D], F32, tag="z2")
                        _ln_apply(nc.vector, rs[i][:], lns2[i], z2[:])
                        nc.vector.tensor_mul(o[:], z2[:], g1b[:])
                        nc.vector.tensor_add(o[:], o[:], be1b[:])
                        nc.sync.dma_start(od, o[:])

            # ---- emission order = near-execution order per engine ----
            st_chunk(0)
            d0 = tree(0)
            denoms(0, d0)
            a0, l0 = mha_stats([0, 1, 2, 3])
            st_chunk(1)
            d1 = tree(1)
            denoms(1, d1)
            apply_xpose([0, 1, 2, 3], a0, l0)
            a1, l1 = mha_stats([4, 5, 6, 7])
            apply_xpose([4, 5, 6, 7], a1, l1)
            ff1_chunk(0)
            r01, l01 = ff2_stats([0, 1])
            ln2_out([0, 1], r01, l01)
            r23, l23 = ff2_stats([2, 3])
            ln2_out([2, 3], r23, l23)
            ff1_chunk(1)
            r45, l45 = ff2_stats([4, 5])
            ln2_out([4, 5], r45, l45)
            r6, l6 = ff2_stats([6])
            ln2_out([6], r6, l6, split=True)
            r7, l7 = ff2_stats([7])
            ln2_out([7], r7, l7, split=True)

    nc.compile()
    _BUILT[fast] = (nc,)
    return _BUILT[fast]


def _ln_stats(nc, small, ap, epsT):
    """One-pass LN stats on DVE: bn_stats/bn_aggr give (mean, var);
    rstd = 1/sqrt(var + eps).  Returns (-mean, rstd) [128,1] tiles.
    Mean is negated so the apply can use the DVE add+mult fast path
    (op0=subtract falls off the fast uop table)."""
    bns = small.tile([128, 6], F32, tag="bns")
    nc.vector.bn_stats(bns[:], ap)
    mv = small.tile([128, 2], F32, tag="mv")
    nc.vector.bn_aggr(mv[:], bns[:])
    nm = small.tile([128, 1], F32, tag="nm")
    nc.vector.tensor_scalar_mul(nm[:], mv[:, 0:1], -1.0)
    stdv = small.tile([128, 1], F32, tag="stdv")
    nc.scalar.activation(stdv[:], mv[:, 1:2], AF.Sqrt, bias=epsT[:])
    rstd = small.tile([128, 1], F32, tag="rstd")
    nc.vector.reciprocal(rstd[:], stdv[:])
    return nm, rstd


def _ln_stats_act(nc, small, work, x1, s1, epsT):
    """ACT-lane LN stats: s1 = row-sum from the caller's accum_out;
    sum-of-squares via ACT Square+accum; var = E[x^2] - mean^2."""
    sq = work.tile([128, D], BF16, tag="sq")
    sqs = small.tile([128, 1], F32, tag="sqs")
    nc.scalar.activation(sq[:], x1[:], AF.Square, bias=0.0, accum_out=sqs[:])
    nm = small.tile([128, 1], F32, tag="nm")
    nc.scalar.mul(nm[:], s1[:], -1.0 / D)
    m2e = small.tile([128, 1], F32, tag="m2e")
    nc.vector.tensor_mul(m2e[:], nm[:], nm[:])
    nc.vector.tensor_scalar(m2e[:], m2e[:], -1.0, 1e-5,
                            op0=ALU.mult, op1=ALU.add)
    stdv = small.tile([128, 1], F32, tag="stdv")
    nc.scalar.activation(stdv[:], sqs[:], AF.Sqrt, scale=1.0 / D, bias=m2e[:])
    rstd = small.tile([128, 1], F32, tag="rstd")
    nc.vector.reciprocal(rstd[:], stdv[:])
    return nm, rstd


def _ln_apply(eng, ap, stats, out_ap):
    nm, rstd = stats
    # (x - mean) * rstd as one op with two per-partition scalars
    eng.tensor_scalar(out_ap, ap, nm[:], rstd[:],
                      op0=ALU.add, op1=ALU.mult)


def _prepare_in_maps(inputs):
    f64 = np.float64
    bf = ml_dtypes.bfloat16
    g = {k: np.asarray(v) for k, v in inputs.items()}
    x = g["x"].astype(f64)
    Wq, Wk, Wv = g["Wq"].astype(f64), g["Wk"].astype(f64), g["Wv"].astype(f64)
    Wo, W1, W2 = g["Wo"].astype(f64), g["W1"].astype(f64), g["W2"].astype(f64)
    bq, bk, bv, bo = g["bq"].astype(f64), g["bk"].astype(f64), g["bv"].astype(f64), g["bo"].astype(f64)
    b1, b2 = g["b1"].astype(f64), g["b2"].astype(f64)
    g0, be0, g1, be1 = g["g0"].astype(f64), g["be0"].astype(f64), g["g1"].astype(f64), g["be1"].astype(f64)

    fast = (
        not np.any(bq) and not np.any(bk) and not np.any(bv) and not np.any(bo)
        and not np.any(b1) and not np.any(b2) and not np.any(be0) and not np.any(be1)
        and bool(np.all(g0 == 1.0)) and bool(np.all(g1 == 1.0))
    )

    s = math.sqrt(D)
    f32 = lambda a: np.ascontiguousarray(a, dtype=np.float32)
    b16 = lambda a: np.ascontiguousarray(a, dtype=np.float32).astype(bf)

    def wlay(W):  # [512, 512] -> SBUF layout [128, bt*512 + n], bf16
        return b16(W.reshape(ND, 128, D).transpose(1, 0, 2).reshape(128, ND * D))

    def wlay_at(W):  # [512, 512] -> [128, at*512 + bt*128 + c], bf16
        return b16(W.reshape(ND, 128, ND, 128).transpose(1, 2, 0, 3)
                   .reshape(128, ND * D))

    Mw = wlay_at(Wq.T @ Wk / s)
    NT = wlay((Wo @ Wv).T)
    W1gT = wlay((W1 * g0[None, :]).T)
    W2T = wlay(W2.T)
    c1 = f32(b1 + W1 @ be0)
    wbo = Wo @ bv + bo
    vk = Wk.T @ bq / s

    shared = dict(Mw=Mw, NT=NT, W1gT=W1gT, W2T=W2T,
                  ident=np.eye(128).astype(bf))
    if not fast:
        shared["C2"] = b16(b2 + be0).reshape(1, D)
        shared["g0b"] = f32(np.broadcast_to(g0, (128, D)))
        shared["g1b"] = f32(np.broadcast_to(g1, (128, D)))
        shared["be1b"] = f32(np.broadcast_to(be1, (128, D)))

    in_maps = []
    for b in range(B):
        xb = x[b]
        m = dict(shared)
        xTf = xb.T.reshape(ND, 128, 4, 256).transpose(2, 1, 0, 3)
        m["xT"] = b16(xTf.reshape(4, 128, ND * 256))
        m["xn"] = b16(xb.reshape(NQ, 128, D).transpose(1, 0, 2).reshape(128, NQ * D))
        smalls = np.zeros((128, 16), np.float32)
        smalls[:, 0:8] = f32(xb @ vk).reshape(8, 128).T
        smalls[:, 8:12] = c1.reshape(4, 128).T
        smalls[:, 12:13] = 1e-5
        m["smalls"] = smalls
        if not fast:
            m["xres"] = f32(xb + wbo[None, :])
        in_maps.append(m)
    return fast, in_maps


def _run(inputs, trace=False):
    fast, in_maps = _prepare_in_maps(inputs)
    (nc,) = _build(fast)
    res = run_bass_kernel_spmd(nc, in_maps, core_ids=list(range(B)), trace=trace)
    out = np.stack([res.results[c]["out"] for c in range(B)]).astype(np.float32)
    return out, res


def kernel(**inputs):
    out, _ = _run(inputs, trace=False)
    return out


# revision 59
# speedup vs baseline: 1.0436x; 1.0327x over previous
"""Trainium2 Bass kernel for nn_Encoder_55362128445616.

Transformer encoder layer: B=8, S=1024, D=512, single-head attention over
H*D=4096. Sharding: data-parallel over batch, one batch element per core,
no collectives.

Key algebraic folding (host-side, exact):
  scores = Q K^T / s = x (Wq^T Wk / s) x^T  (+ per-k bias from bq; per-q
  terms cancel in softmax), so the 4096-dim QKV contractions collapse to
  512-dim ones via M = Wq^T Wk / s and NT = (Wo Wv)^T.  LN1's affine is
  folded into W1/b1.

Perf design (vs the f32r baseline):
  - all matmul data is bf16 (same 1 cyc/row PE throughput as f32r, but
    half the DMA bytes and 2x faster LDWEIGHTS)
  - critical head loads (Mw at-chunks, x^T halves) split across the three
    DMA queues so phase 1 starts as early as possible
  - the z -> zT transpose between LN1 and ff1 runs on the DMA xbar
    (dma_start(transpose=True)) in qt-pairs on alternating queues, not PE
  - LN mean/var via one-pass bn_stats/bn_aggr on DVE; psum evacuations on
    ACT; LN2 applies on Pool (idle at the tail) to shorten the exit chain
"""

import math

import numpy as np
import ml_dtypes

# If the environment sets BASS_TRACE, bass_utils imports antenv.axon_hooks,
# which this image may lack — provide a no-op stub so plain runs never crash.
import sys as _sys
import types as _types
try:
    import antenv.axon_hooks  # noqa: F401
except ImportError:
    _m = _types.ModuleType("antenv.axon_hooks")
    _m.get_axon_ntff_profile_hook = lambda: None
    _m.set_axon_ntff_profile_hook = lambda hook: None
    _sys.modules["antenv.axon_hooks"] = _m

import concourse.bacc as bacc
import concourse.mybir as mybir
import concourse.tile as tile
from concourse.bass_utils import run_bass_kernel_spmd

B, S, D = 8, 1024, 512
NQ = S // 128   # 8 q/k tiles of 128
ND = D // 128   # 4 d tiles of 128
F32 = mybir.dt.float32
BF16 = mybir.dt.bfloat16
AF = mybir.ActivationFunctionType
ALU = mybir.AluOpType
N_WARM = 36

_BUILT = {}


def _build(fast):
    if fast in _BUILT:
        return _BUILT[fast]

    nc = bacc.Bacc("TRN2", target_bir_lowering=False, debug=False, num_devices=B)

    def din(name, shape, dt=BF16):
        return nc.dram_tensor(name, shape, dt, kind="ExternalInput").ap()

    # all big inputs are pre-arranged on host to the exact SBUF layout so
    # every load is a contiguous DMA at max HBM rate
    xT_d = din("xT", [2, 128, ND * 512])  # x[b].T as [q-half][p][bt*512]
    xn_d = din("xn", [128, NQ * D])       # x[b] as [p][kt*D]
    M_d = din("Mw", [128, ND * D])        # Wq^T Wk / sqrt(D), [p][at][bt][c]
    NT_d = din("NT", [128, ND * D])       # [p][bt][n]
    W1gT_d = din("W1gT", [128, ND * D])
    W2T_d = din("W2T", [128, ND * D])
    # packed per-partition columns: [0:8]=abias, [8:12]=c1, [12:13]=eps
    sm_d = din("smalls", [128, 16], F32)
    id_d = din("ident", [128, 128])       # bf16 identity for psum residual
    if not fast:
        xres_d = din("xres", [S, D], F32)     # x[b] + (Wo@bv + bo)
        C2_d = din("C2", [1, D])              # b2 + be0 (bf16)
        g0b_d = din("g0b", [128, D], F32)
        g1b_d = din("g1b", [128, D], F32)
        be1b_d = din("be1b", [128, D], F32)
    out_d = nc.dram_tensor("out", [S, D], F32, kind="ExternalOutput").ap()

    with tile.TileContext(nc) as tc:
        with (
            tc.tile_pool(name="res", bufs=1) as res,
            tc.tile_pool(name="work", bufs=2) as work,
            tc.tile_pool(name="small", bufs=8) as small,
            tc.tile_pool(name="psA", bufs=4, space="PSUM") as psA,
            tc.tile_pool(name="psS", bufs=2, space="PSUM") as psS,
        ):
            # ---- resident loads, split across the three DMA queues so the
            # phase-1 critical path (Mw at01 + xT h0) lands in parallel ----
            xT = res.tile([128, ND, S], BF16)
            Mw = res.tile([128, ND, ND, 128], BF16)   # [p][at][bt][c]
            Mv = M_d.rearrange("p (a t c) -> p a t c", t=ND, c=128)

            # sync: h0 bt01 then h1 bt01; later: transposes + out stores
            nc.sync.dma_start(xT[:, 0:2, 0:512], xT_d[0].rearrange(
                "p (t q) -> p t q", q=512)[:, 0:2, :])
            # scalar: Mw at01 then h1 bt23; later: psum evacuations
            nc.scalar.dma_start(Mw[:, 0:2], Mv[:, 0:2])
            # gpsimd: h0 bt23, Mw at23, then bulk weights
            nc.gpsimd.dma_start(xT[:, 2:4, 0:512], xT_d[0].rearrange(
                "p (t q) -> p t q", q=512)[:, 2:4, :])
            nc.sync.dma_start(xT[:, 0:2, 512:1024], xT_d[1].rearrange(
                "p (t q) -> p t q", q=512)[:, 0:2, :])
            nc.scalar.dma_start(xT[:, 2:4, 512:1024], xT_d[1].rearrange(
                "p (t q) -> p t q", q=512)[:, 2:4, :])
            nc.gpsimd.dma_start(Mw[:, 2:4], Mv[:, 2:4])
            sm = res.tile([128, 16], F32)
            nc.scalar.dma_start(sm[:], sm_d[:])
            ident = res.tile([128, 128], BF16)
            nc.scalar.dma_start(ident[:], id_d[:])
            xn = res.tile([128, NQ, D], BF16)
            nc.gpsimd.dma_start(xn[:], xn_d.rearrange("p (t n) -> p t n", n=D))
            NTw = res.tile([128, ND, D], BF16)
            nc.gpsimd.dma_start(NTw[:], NT_d.rearrange("p (t n) -> p t n", n=D))
            W1gT = res.tile([128, ND, D], BF16)
            nc.scalar.dma_start(W1gT[:], W1gT_d.rearrange("p (t n) -> p t n", n=D))
            W2T = res.tile([128, ND, D], BF16)
            nc.scalar.dma_start(W2T[:], W2T_d.rearrange("p (t n) -> p t n", n=D))
            if not fast:
                xres = res.tile([128, NQ, D], F32)
                nc.gpsimd.dma_start(xres[:], xres_d.rearrange("(t p) n -> p t n", p=128))
                C2 = res.tile([1, D], BF16)
                nc.gpsimd.dma_start(C2[:], C2_d[:])
                g0b = res.tile([128, D], F32)
                nc.gpsimd.dma_start(g0b[:], g0b_d[:])
                g1b = res.tile([128, D], F32)
                nc.gpsimd.dma_start(g1b[:], g1b_d[:])
                be1b = res.tile([128, D], F32)
                nc.gpsimd.dma_start(be1b[:], be1b_d[:])

            abias = sm[:, 0:8]
            c1 = sm[:, 8:12]
            epsT = sm[:, 12:13]

            # device-built constants (no DMA)
            onesb = res.tile([128, 2], BF16)
            nc.vector.memset(onesb[:], 1.0)
            if not fast:
                onesr = res.tile([1, 128], BF16)
                nc.vector.memset(onesr[:], 1.0)

            # HAM warm-up on an uninitialized scratch tile: no input deps, so
            # the PE starts (and its p-state ramp clock) while DMAs land.
            wtile = res.tile([128, 128], BF16)
            nc.vector.memset(wtile[:], 0.0)
            for w in range(N_WARM):
                psw = psA.tile([128, 128], F32, tag="a", name=f"psw{w}")
                nc.tensor.matmul(psw[:], wtile[:], wtile[:], start=True, stop=True)

            # big SBUF intermediates (all bf16)
            IN2 = res.tile([128, ND, S], BF16)      # (x M)^T
            PT = res.tile([128, NQ, S], BF16)       # exp(scores^T)
            ST = res.tile([128, ND, S], BF16)       # sdpa^T
            # zT in qt-major blocks: [p][qt][dt][c]; a qt-pair DMA-transpose
            # writes one contiguous [128, 1024] span
            zT = res.tile([128, NQ, ND, 128], BF16)
            ff1T = res.tile([128, ND, S], BF16)
            z = res.tile([128, NQ, D], BF16)        # LN1 out
            recip = res.tile([128, 2 * NQ], F32)

            # ---- phase 1: IN2[a, q] = sum_b M[b, a] xT[b, q] ----
            # 512-column chunks (matmuls have a ~213ns instruction floor on
            # HW, so smaller chunks don't pay); the quartered loads mean the
            # qc0 half only waits for x^T quarters 0+1
            for qc in range(2):
                for at in range(ND):
                    ps = psA.tile([128, 512], F32, tag="a", name="ps_p1")
                    for bt in range(ND):
                        nc.tensor.matmul(
                            ps[:],
                            Mw[:, at, bt, :],
                            xT[:, bt, qc * 512:(qc + 1) * 512],
                            start=(bt == 0), stop=(bt == ND - 1),
                        )
                    nc.scalar.copy(IN2[:, at, qc * 512:(qc + 1) * 512], ps[:])

            # ---- phase 2: scoresT[k, q] = sum_a x[k, a] IN2[a, q]; PT = exp ----
            for kt in range(NQ):
                ps = psS.tile([128, 1024], F32, tag="s")
                for qc in range(2):
                    for at in range(ND):
                        nc.tensor.matmul(
                            ps[:, qc * 512:(qc + 1) * 512],
                            xT[:, at, kt * 128:(kt + 1) * 128],
                            IN2[:, at, qc * 512:(qc + 1) * 512],
                            start=(at == 0), stop=(at == ND - 1),
                        )
                bias = 0.0 if fast else abias[:, kt:kt + 1]
                nc.scalar.activation(PT[:, kt, :], ps[:], AF.Exp, bias=bias)

            # softmax denominator: DVE tree-sums the 8 k-tiles (pairwise),
            # then one tiny ones-matmul per q-tile flips [k-part, q] to
            # [q-part, 1].
            def tree(qc):
                qs = slice(qc * 512, (qc + 1) * 512)
                lvl1 = []
                for i in range(4):
                    t = work.tile([128, 512], BF16, tag="tr", bufs=6, name=f"t{qc}{i}")
                    nc.vector.tensor_add(t[:], PT[:, 2 * i, qs], PT[:, 2 * i + 1, qs])
                    lvl1.append(t)
                u0 = work.tile([128, 512], BF16, tag="tr", bufs=6, name=f"u{qc}0")
                nc.vector.tensor_add(u0[:], lvl1[0][:], lvl1[1][:])
                u1 = work.tile([128, 512], BF16, tag="tr", bufs=6, name=f"u{qc}1")
                nc.vector.tensor_add(u1[:], lvl1[2][:], lvl1[3][:])
                dacc = work.tile([128, 512], BF16, tag="dacc", bufs=2, name=f"dacc{qc}")
                nc.vector.tensor_add(dacc[:], u0[:], u1[:])
                return dacc

            def denoms(qc, dacc):
                dps = psA.tile([128, 512], F32, tag="a", name=f"dps{qc}")
                for ql in range(4):
                    nc.tensor.matmul(
                        dps[:, 2 * ql:2 * ql + 2],
                        dacc[:, ql * 128:(ql + 1) * 128],
                        onesb[:],
                        start=True, stop=True,
                    )
                nc.vector.reciprocal(recip[:, qc * 8:(qc + 1) * 8], dps[:, 0:8])

            # ---- phase 3: ST[d, q] = sum_k x[k, d] PT[k, q] ----
            def st_chunk(qc):
                for dt in range(ND):
                    ps = psA.tile([128, 512], F32, tag="a", name="ps_st")
                    for kt in range(NQ):
                        nc.tensor.matmul(
                            ps[:],
                            xn[:, kt, dt * 128:(dt + 1) * 128],
                            PT[:, kt, qc * 512:(qc + 1) * 512],
                            start=(kt == 0), stop=(kt == NQ - 1),
                        )
                    nc.scalar.copy(ST[:, dt, qc * 512:(qc + 1) * 512], ps[:])

            # ---- phase 4: mha + residual + LN1 stats per q-tile ----
            def mha_stats(qts):
                x1s, lns = [], []
                for qt in qts:
                    ps = psA.tile([128, 512], F32, tag="a", name="ps_mha")
                    for dt in range(ND):
                        nc.tensor.matmul(
                            ps[:],
                            ST[:, dt, qt * 128:(qt + 1) * 128],
                            NTw[:, dt, :],
                            start=(dt == 0), stop=(dt == ND - 1),
                        )
                    x1 = work.tile([128, D], BF16, tag="x1", bufs=NQ, name=f"x1_{qt}")
                    resid = xn[:, qt, :] if fast else xres[:, qt, :]
                    # alternate the stats chain between DVE (bn_stats) and
                    # ACT (square+accum) lanes so neither engine saturates;
                    # qt>=4 all on ACT so the DVE applies (which gate the
                    # zT transposes for ff1 qc1) clear sooner
                    on_act = qt % 2 == 1 or qt >= 4
                    s1 = (small.tile([128, 1], F32, tag="s1", name=f"s1_{qt}")
                          if on_act else None)
                    nc.vector.scalar_tensor_tensor(
                        x1[:], ps[:], recip[:, 2 * qt:2 * qt + 1], resid,
                        op0=ALU.mult, op1=ALU.add,
                        accum_out=s1[:] if on_act else None,
                    )
                    x1s.append(x1)
                    if on_act:
                        lns.append(_ln_stats_act(nc, small, work, x1, s1, epsT))
                    else:
                        lns.append(_ln_stats(nc, small, x1[:], epsT))
                return x1s, lns

            # ---- LN1 apply; DMA-xbar transpose z -> zT in qt pairs ----
            def apply_xpose(qts, x1s, lns):
                for i, qt in enumerate(qts):
                    _ln_apply(nc.vector, x1s[i][:], lns[i], z[:, qt, :])
                    # all transposes on the sync queue: it is idle mid-body,
                    # while a transpose on the scalar queue would block the
                    # ACT engine behind it (psum evacuations, relu)
                    if qt % 2 == 1:
                        q0 = qt - 1
                        nc.sync.dma_start(
                            zT[:, q0:q0 + 2], z[:, q0:q0 + 2, :],
                            transpose=True)

            # ---- phase 5: ff1 ----
            def ff1_chunk(qc):
                for et in range(ND):
                    ps = psA.tile([128, 512], F32, tag="a", name="ps_ff1")
                    for dt in range(ND):
                        nc.tensor.matmul(
                            ps[:],
                            W1gT[:, dt, et * 128:(et + 1) * 128],
                            zT[:, qc * 4:(qc + 1) * 4, dt, :],
                            start=(dt == 0), stop=(dt == ND - 1),
                        )
                    bias = 0.0 if fast else c1[:, et:et + 1]
                    nc.scalar.activation(
                        ff1T[:, et, qc * 512:(qc + 1) * 512], ps[:],
                        AF.Relu, bias=bias,
                    )

            # ---- phase 6: ff2 + residual + LN2 stats ----
            # fast path: the z residual is added INTO the psum by one extra
            # identity matmul, so stats and apply read the psum directly —
            # no DVE stt at all (and f32 residual precision for free)
            def ff2_stats(qts):
                rs, lns2 = [], []
                if fast:
                    # interleave the q-tiles' et-accumulations so the last
                    # relu evacuation's latency is covered by the other
                    # tile's matmuls (removes the ff1->ff2 boundary stall)
                    pss = [psA.tile([128, 512], F32, tag="a", name=f"ps_ff2_{qt}")
                           for qt in qts]
                    for et in range(ND):
                        for i, qt in enumerate(qts):
                            nc.tensor.matmul(
                                pss[i][:],
                                ff1T[:, et, qt * 128:(qt + 1) * 128],
                                W2T[:, et, :],
                                start=(et == 0), stop=False,
                            )
                    for i, qt in enumerate(qts):
                        nc.tensor.matmul(pss[i][:], ident[:], z[:, qt, :],
                                         start=False, stop=True)
                    for i, qt in enumerate(qts):
                        rs.append(pss[i])
                        lns2.append(_ln_stats(nc, small, pss[i][:], epsT))
                    return rs, lns2
                for qt in qts:
                    ps = psA.tile([128, 512], F32, tag="a", name="ps_ff2")
                    for et in range(ND):
                        nc.tensor.matmul(
                            ps[:],
                            ff1T[:, et, qt * 128:(qt + 1) * 128],
                            W2T[:, et, :],
                            start=(et == 0), stop=False,
                        )
                    if True:
                        nc.tensor.matmul(ps[:], onesr[:], C2[:], start=False, stop=True)
                        r = work.tile([128, D], BF16, tag="r", bufs=NQ, name=f"r_{qt}")
                        hres = work.tile([128, D], F32, tag="hres")
                        nc.vector.tensor_mul(hres[:], z[:, qt, :], g0b[:])
                        nc.vector.scalar_tensor_tensor(
                            r[:], ps[:], 1.0, hres[:],
                            op0=ALU.mult, op1=ALU.add,
                        )
                        rs.append(r)
                        lns2.append(_ln_stats(nc, small, r[:], epsT))
                return rs, lns2

            def ln2_out(qts, rs, lns2, split=False):
                for i, qt in enumerate(qts):
                    od = out_d.rearrange("(t p) n -> p t n", p=128)[:, qt, :]
                    o = work.tile([128, D], F32, tag="o", bufs=3, name=f"o_{qt}")
                    # all stores go on the sync queue: a store issued on the
                    # scalar queue blocks the ACT engine behind it, delaying
                    # the tail sqrt chains
                    if fast and split:
                        # halves pipeline the apply with the store at the tail
                        for h in range(2):
                            hs = slice(h * 256, (h + 1) * 256)
                            _ln_apply(nc.vector, rs[i][:, hs], lns2[i], o[:, hs])
                            nc.sync.dma_start(od[:, hs], o[:, hs])
                    elif fast:
                        _ln_apply(nc.vector, rs[i][:], lns2[i], o[:])
                        nc.sync.dma_start(od, o[:])
                    else:
                        z2 = work.tile([128, D], F32, tag="z2")
                        _ln_apply(nc.vector, rs[i][:], lns2[i], z2[:])
                        nc.vector.tensor_mul(o[:], z2[:], g1b[:])
                        nc.vector.tensor_add(o[:], o[:], be1b[:])
                        nc.sync.dma_start(od, o[:])

            # ---- emission order = near-execution order per engine ----
            st_chunk(0)
            d0 = tree(0)
            denoms(0, d0)
            a0, l0 = mha_stats([0, 1, 2, 3])
            st_chunk(1)
            d1 = tree(1)
            denoms(1, d1)
            apply_xpose([0, 1, 2, 3], a0, l0)
            a1, l1 = mha_stats([4, 5, 6, 7])
            apply_xpose([4, 5, 6, 7], a1, l1)
            ff1_chunk(0)
            r01, l01 = ff2_stats([0, 1])
            ln2_out([0, 1], r01, l01)
            r23, l23 = ff2_stats([2, 3])
            ln2_out([2, 3], r23, l23)
            ff1_chunk(1)
            r45, l45 = ff2_stats([4, 5])
            ln2_out([4, 5], r45, l45)
            r6, l6 = ff2_stats([6])
            ln2_out([6], r6, l6, split=True)
            r7, l7 = ff2_stats([7])
            ln2_out([7], r7, l7, split=True)

    nc.compile()
    _BUILT[fast] = (nc,)
    return _BUILT[fast]


def _ln_stats(nc, small, ap, epsT):
    """One-pass LN stats on DVE: bn_stats/bn_aggr give (mean, var);
    rstd = 1/sqrt(var + eps).  Returns (-mean, rstd) [128,1] tiles.
    Mean is negated so the apply can use the DVE add+mult fast path
    (op0=subtract falls off the fast uop table)."""
    bns = small.tile([128, 6], F32, tag="bns")
    nc.vector.bn_stats(bns[:], ap)
    mv = small.tile([128, 2], F32, tag="mv")
    nc.vector.bn_aggr(mv[:], bns[:])
    nm = small.tile([128, 1], F32, tag="nm")
    nc.vector.tensor_scalar_mul(nm[:], mv[:, 0:1], -1.0)
    stdv = small.tile([128, 1], F32, tag="stdv")
    nc.scalar.activation(stdv[:], mv[:, 1:2], AF.Sqrt, bias=epsT[:])
    rstd = small.tile([128, 1], F32, tag="rstd")
    nc.vector.reciprocal(rstd[:], stdv[:])
    return nm, rstd


def _ln_stats_act(nc, small, work, x1, s1, epsT):
    """ACT-lane LN stats: s1 = row-sum from the caller's accum_out;
    sum-of-squares via ACT Square+accum; var = E[x^2] - mean^2."""
    sq = work.tile([128, D], BF16, tag="sq")
    sqs = small.tile([128, 1], F32, tag="sqs")
    nc.scalar.activation(sq[:], x1[:], AF.Square, bias=0.0, accum_out=sqs[:])
    nm = small.tile([128, 1], F32, tag="nm")
    nc.scalar.mul(nm[:], s1[:], -1.0 / D)
    m2e = small.tile([128, 1], F32, tag="m2e")
    nc.vector.tensor_mul(m2e[:], nm[:], nm[:])
    nc.vector.tensor_scalar(m2e[:], m2e[:], -1.0, 1e-5,
                            op0=ALU.mult, op1=ALU.add)
    stdv = small.tile([128, 1], F32, tag="stdv")
    nc.scalar.activation(stdv[:], sqs[:], AF.Sqrt, scale=1.0 / D, bias=m2e[:])
    rstd = small.tile([128, 1], F32, tag="rstd")
    nc.vector.reciprocal(rstd[:], stdv[:])
    return nm, rstd


def _ln_apply(eng, ap, stats, out_ap):
    nm, rstd = stats
    # (x - mean) * rstd as one op with two per-partition scalars
    eng.tensor_scalar(out_ap, ap, nm[:], rstd[:],
                      op0=ALU.add, op1=ALU.mult)


def _prepare_in_maps(inputs):
    f64 = np.float64
    bf = ml_dtypes.bfloat16
    g = {k: np.asarray(v) for k, v in inputs.items()}
    x = g["x"].astype(f64)
    Wq, Wk, Wv = g["Wq"].astype(f64), g["Wk"].astype(f64), g["Wv"].astype(f64)
    Wo, W1, W2 = g["Wo"].astype(f64), g["W1"].astype(f64), g["W2"].astype(f64)
    bq, bk, bv, bo = g["bq"].astype(f64), g["bk"].astype(f64), g["bv"].astype(f64), g["bo"].astype(f64)
    b1, b2 = g["b1"].astype(f64), g["b2"].astype(f64)
    g0, be0, g1, be1 = g["g0"].astype(f64), g["be0"].astype(f64), g["g1"].astype(f64), g["be1"].astype(f64)

    fast = (
        not np.any(bq) and not np.any(bk) and not np.any(bv) and not np.any(bo)
        and not np.any(b1) and not np.any(b2) and not np.any(be0) and not np.any(be1)
        and bool(np.all(g0 == 1.0)) and bool(np.all(g1 == 1.0))
    )

    s = math.sqrt(D)
    f32 = lambda a: np.ascontiguousarray(a, dtype=np.float32)
    b16 = lambda a: np.ascontiguousarray(a, dtype=np.float32).astype(bf)

    def wlay(W):  # [512, 512] -> SBUF layout [128, bt*512 + n], bf16
        return b16(W.reshape(ND, 128, D).transpose(1, 0, 2).reshape(128, ND * D))

    def wlay_at(W):  # [512, 512] -> [128, at*512 + bt*128 + c], bf16
        return b16(W.reshape(ND, 128, ND, 128).transpose(1, 2, 0, 3)
                   .reshape(128, ND * D))

    Mw = wlay_at(Wq.T @ Wk / s)
    NT = wlay((Wo @ Wv).T)
    W1gT = wlay((W1 * g0[None, :]).T)
    W2T = wlay(W2.T)
    c1 = f32(b1 + W1 @ be0)
    wbo = Wo @ bv + bo
    vk = Wk.T @ bq / s

    shared = dict(Mw=Mw, NT=NT, W1gT=W1gT, W2T=W2T,
                  ident=np.eye(128).astype(bf))
    if not fast:
        shared["C2"] = b16(b2 + be0).reshape(1, D)
        shared["g0b"] = f32(np.broadcast_to(g0, (128, D)))
        shared["g1b"] = f32(np.broadcast_to(g1, (128, D)))
        shared["be1b"] = f32(np.broadcast_to(be1, (128, D)))

    in_maps = []
    for b in range(B):
        xb = x[b]
        m = dict(shared)
        xTf = xb.T.reshape(ND, 128, 2, 512).transpose(2, 1, 0, 3)
        m["xT"] = b16(xTf.reshape(2, 128, ND * 512))
        m["xn"] = b16(xb.reshape(NQ, 128, D).transpose(1, 0, 2).reshape(128, NQ * D))
        smalls = np.zeros((128, 16), np.float32)
        smalls[:, 0:8] = f32(xb @ vk).reshape(8, 128).T
        smalls[:, 8:12] = c1.reshape(4, 128).T
        smalls[:, 12:13] = 1e-5
        m["smalls"] = smalls
        if not fast:
            m["xres"] = f32(xb + wbo[None, :])
        in_maps.append(m)
    return fast, in_maps


def _run(inputs, trace=False):
    fast, in_maps = _prepare_in_maps(inputs)
    (nc,) = _build(fast)
    res = run_bass_kernel_spmd(nc, in_maps, core_ids=list(range(B)), trace=trace)
    out = np.stack([res.results[c]["out"] for c in range(B)]).astype(np.float32)
    return out, res


def kernel(**inputs):
    out, _ = _run(inputs, trace=False)
    return out
